# revision 4
# baseline (speedup 1.0000x reference)
"""CTC-style loss (nn_CTCFormal) on 8 Trainium2 NeuronCores.

Pure data parallel over batch N=4096 -> 512 samples/core (128 partitions x
4 groups).  The alpha DP runs in blank-ratio-normalized form: dividing
alpha by the running blank product B_t = prod_t y_blank[t] turns the
63-state blank-interleaved recurrence into two 32-col channels
  b~[j] (blank states s=2j)  and  l~[j] (label states s=2j+1)
with per-step updates
  b~ += shift1(l~)                  (no blank multiply at all)
  u   = l~ + b~'                    (the skip term sk[j]*l~[j-1] is already
                                     inside b~' when labels don't repeat;
                                     repeats get a small correction op)
  l~  = u * ytil[t]                 (ytil = exp(lp_label - lp_blank), bf16)
and the host finishes with loss_n = -ln(b~[31]+l~[30]) - sum_t lp_blank[t]
summed in fp64.

Performance structure (measured on this silicon):
- bf16 contiguous tensor_tensor runs in 2x mode: ~(FD/2 + 66) DVE cycles.
- A dependent op pays a ~90-cycle SBUF read-after-write stall; two
  independent chains (two disjoint sample halves, fully separate tiles)
  interleaved op-by-op give every op RAW-distance 2 and hide the stall.
- Concurrent DMA degrades 2x-mode ops to 1x AND the DMA itself crawls
  (~40-75 GB/s vs ~300 solo), so the ytil load is fully serialized
  before the loop, split across both HWDGE rings (sync + scalar).
- The DP diamond (alpha support) lets each step run on a j-window
  [max(0,t-33), min(t,..)+1) -- ~45% less element work.
"""

import numpy as np

T, N, C = 64, 4096, 128
L = 31
NCORES = 8
NLOC = N // NCORES  # 512
P = 128
G = NLOC // P  # 4
GC = G // 2  # groups per chain

_BASS_CACHE = {}


def _build_bass(corr_groups):
    key = ("nc", corr_groups)
    if key in _BASS_CACHE:
        return _BASS_CACHE[key]

    import concourse.bacc as bacc
    import concourse.mybir as mybir
    from concourse.tile import TileContext

    f32 = mybir.dt.float32
    bf16 = mybir.dt.bfloat16

    nc = bacc.Bacc(trn_type="TRN2")
    ya_d = nc.declare_dram_parameter("yta", [P, T, GC, 32], bf16, isOutput=False)
    yb_d = nc.declare_dram_parameter("ytb", [P, T, GC, 32], bf16, isOutput=False)
    l0_d = nc.declare_dram_parameter("l0", [P, G], bf16, isOutput=False)
    nsk_d = nc.declare_dram_parameter("nskt", [P, G, 32], bf16, isOutput=False)
    r_d = nc.declare_dram_parameter("r", [P, G], bf16, isOutput=True)

    with TileContext(nc) as tc:
        with tc.tile_pool(name="main", bufs=1) as pool:
            yA = pool.tile([P, T, GC, 32], bf16)
            yB = pool.tile([P, T, GC, 32], bf16)
            lA = pool.tile([P, GC, 32], bf16)  # col 0 zero-slot, label jj at col 1+jj
            lB = pool.tile([P, GC, 32], bf16)
            bA = pool.tile([P, GC, 32], bf16)  # blank j at col j
            bB = pool.tile([P, GC, 32], bf16)
            uA = pool.tile([P, GC, 32], bf16)
            uB = pool.tile([P, GC, 32], bf16)
            nskt = pool.tile([P, G, 32], bf16)
            cw = pool.tile([P, 32], bf16)

            # serialized input load, split across both HWDGE rings
            nc.sync.dma_start(out=yA[:], in_=ya_d[:])
            nc.scalar.dma_start(out=yB[:], in_=yb_d[:])
            if corr_groups:
                nc.sync.dma_start(out=nskt[:], in_=nsk_d[:])

            for tl, ch in ((lA, 0), (lB, 1)):
                nc.vector.memset(tl[:], 0.0)
                nc.sync.dma_start(
                    out=tl[:, :, 1:2], in_=l0_d[:, ch * GC : (ch + 1) * GC]
                )
            for tl in (bA, bB):
                nc.vector.memset(tl[:], 0.0)
                nc.vector.memset(tl[:, :, 0:1], 1.0)

            # correction ops go on the chain(s) owning the dirty groups
            # (dirty samples are permuted to the top groups = chain B first)
            dirty = []  # (u-tile, nsk-row, l-tile, local group)
            for k in range(corr_groups):
                g = G - 1 - k
                ch = g // GC
                dirty.append((uB if ch else uA, g, lB if ch else lA, g % GC))

            for t in range(1, T):
                jlo = max(0, t - 33)
                jhb = min(t, 31) + 1
                jhu = min(t, 30) + 1
                # op1: b~[j] += l~[j-1]   (l-tile col j holds l~[j-1])
                nc.vector.tensor_add(
                    out=bA[:, :, jlo:jhb], in0=bA[:, :, jlo:jhb], in1=lA[:, :, jlo:jhb]
                )
                nc.vector.tensor_add(
                    out=bB[:, :, jlo:jhb], in0=bB[:, :, jlo:jhb], in1=lB[:, :, jlo:jhb]
                )
                # op2: u[j] = l~[j] + b~'[j]
                nc.vector.tensor_add(
                    out=uA[:, :, jlo:jhu],
                    in0=lA[:, :, jlo + 1 : jhu + 1],
                    in1=bA[:, :, jlo:jhu],
                )
                nc.vector.tensor_add(
                    out=uB[:, :, jlo:jhu],
                    in0=lB[:, :, jlo + 1 : jhu + 1],
                    in1=bB[:, :, jlo:jhu],
                )
                # repeat-label correction: u[g,jj] -= nsk[g,jj] * l~[jj-1]
                for ut, g, ltile, gl in dirty:
                    nc.vector.tensor_mul(
                        out=cw[:, jlo:jhu],
                        in0=nskt[:, g, jlo:jhu],
                        in1=ltile[:, gl, jlo:jhu],
                    )
                    nc.vector.tensor_sub(
                        out=ut[:, gl, jlo:jhu],
                        in0=ut[:, gl, jlo:jhu],
                        in1=cw[:, jlo:jhu],
                    )
                # op3: l~[jj] = u[jj] * ytil[t, jj]
                nc.vector.tensor_mul(
                    out=lA[:, :, jlo + 1 : jhu + 1],
                    in0=uA[:, :, jlo:jhu],
                    in1=yA[:, t, :, jlo:jhu],
                )
                nc.vector.tensor_mul(
                    out=lB[:, :, jlo + 1 : jhu + 1],
                    in0=uB[:, :, jlo:jhu],
                    in1=yB[:, t, :, jlo:jhu],
                )

            # r = b~[j=31] + l~[jj=30]  (alpha_T[62] + alpha_T[61]) / B_T
            rb = pool.tile([P, G], bf16)
            nc.vector.tensor_add(
                out=rb[:, 0:GC], in0=bA[:, :, 31], in1=lA[:, :, 31]
            )
            nc.vector.tensor_add(
                out=rb[:, GC:G], in0=bB[:, :, 31], in1=lB[:, :, 31]
            )
            nc.sync.dma_start(out=r_d[:], in_=rb[:])

    nc.finalize()
    _BASS_CACHE[key] = nc
    return nc


def host_prep(input, target, input_length, target_length):
    import ml_dtypes

    bf = ml_dtypes.bfloat16
    inp = np.asarray(input, dtype=np.float32)
    target = np.asarray(target, dtype=np.int32)
    tl = np.asarray(target_length, dtype=np.int64)

    # reference's buggy padding: start_i = target_length[i-1] if i>0 else 0,
    # clamped like jax.lax.dynamic_slice
    starts = np.zeros(N, np.int64)
    starts[1:] = tl[: N - 1]
    starts = np.clip(starts, 0, len(target) - L)
    lab = target[starts[:, None] + np.arange(L)]  # [N, L]

    nsk = np.zeros((N, L), np.float32)
    nsk[:, 1:] = (lab[:, 1:] == lab[:, :-1]).astype(np.float32)
    dirty = nsk.sum(1) > 0
    n_dirty = int(dirty.sum())

    # spread dirty samples evenly across cores, placed in the top groups
    order = np.argsort(dirty, kind="stable")  # clean first
    clean_ids = order[: N - n_dirty]
    dirty_ids = order[N - n_dirty :]
    perm = np.empty(N, np.int64)
    pos = np.zeros(NCORES, np.int64)
    for i, n in enumerate(dirty_ids):
        c = i % NCORES
        perm[c * NLOC + NLOC - 1 - pos[c]] = n
        pos[c] += 1
    fill = np.zeros(NCORES, np.int64)
    slot = 0
    for n in clean_ids:
        while fill[slot % NCORES] >= NLOC - pos[slot % NCORES]:
            slot += 1
        c = slot % NCORES
        perm[c * NLOC + fill[c]] = n
        fill[c] += 1
        slot += 1
    corr_groups = min(G, -(-int(pos.max()) // P)) if n_dirty else 0

    lp = inp.transpose(1, 2, 0)  # [N, C, T]
    lpb_total = float(lp[:, 0, :].astype(np.float64).sum())
    lpl = np.take_along_axis(lp, lab[:, :, None].astype(np.int64), axis=1)
    ytil = np.exp(lpl - lp[:, 0:1, :]).astype(np.float32)  # [N, L, T]

    in_maps = []
    for c in range(NCORES):
        ids = perm[c * NLOC : (c + 1) * NLOC]
        y = np.zeros((P, T, G, 32), np.float32)
        # sample s0 -> partition s0 % P, group s0 // P
        yv = ytil[ids].transpose(0, 2, 1).reshape(G, P, T, L)
        for g in range(G):
            y[:, :, g, 0:L] = yv[g]
        nk = np.zeros((P, G, 32), np.float32)
        nkv = nsk[ids].reshape(G, P, L)
        for g in range(G):
            nk[:, g, 0:L] = nkv[g]
        yc = y.astype(bf)
        in_maps.append(
            {
                "yta": np.ascontiguousarray(yc[:, :, 0:GC]),
                "ytb": np.ascontiguousarray(yc[:, :, GC:G]),
                "l0": np.ascontiguousarray(yc[:, 0, :, 0]),
                "nskt": np.ascontiguousarray(nk.astype(bf)),
            }
        )
    return in_maps, corr_groups, lpb_total


def kernel(input, target, input_length, target_length):
    from concourse.bass_utils import run_bass_kernel_spmd

    in_maps, corr_groups, lpb_total = host_prep(
        input, target, input_length, target_length
    )
    nc = _build_bass(corr_groups)
    res = run_bass_kernel_spmd(nc, in_maps, list(range(NCORES)))
    total = -lpb_total
    for core in range(NCORES):
        r = np.asarray(res.results[core]["r"], dtype=np.float64)
        total -= float(np.log(r).sum())
    return np.float32(total)
